# revision 14
# baseline (speedup 1.0000x reference)
import numpy as np
import ml_dtypes

N=4096; C=1024; INTER=128; R=128; GC=256; NCORES=8; NB=N//NCORES
HR=R//NCORES; PW=130; WINR=HR+2; WIN=WINR*PW; QT=19; KT=C//128
GXE=NB*INTER; BOFF=GXE; QOFF=GXE+NB; ZOFF=QOFF+2304; AGE=QOFF+2432  # 68480
BF=ml_dtypes.bfloat16

_cache = {}

def _fold(p):
    f32=np.float32
    out={}
    mcw1=np.asarray(p['m_cw'][:INTER],f32); mcw2=np.asarray(p['m_cw'][INTER:],f32)
    xv=np.zeros((C,6),f32); sc=np.zeros((1,8),f32)
    xv[:,0]=np.asarray(p['m_tw'],f32).T@mcw1; sc[0,0]=np.asarray(p['m_tb'],f32)@mcw1
    for j in range(3):
        c1=np.asarray(p['pr_cw'][j,:INTER],f32); c2=np.asarray(p['pr_cw'][j,INTER:],f32)
        xv[:,1+j]=np.asarray(p['pr_tw'][j],f32).T@c1
        sc[0,1+j]=np.asarray(p['pr_tb'][j],f32)@c1+np.asarray(p['pr_pb'][j],f32)@c2
    bc1=np.asarray(p['ba_cw'][:INTER],f32); bc2=np.asarray(p['ba_cw'][INTER:],f32)
    xv[:,4]=np.asarray(p['ba_tw'],f32).T@bc1
    xv[:,5]=np.asarray(p['m_pw'],f32).T@mcw2; sc[0,5]=np.asarray(p['m_pb'],f32)@mcw2
    # bef/aft enter only via column means: fold on host
    cs=(np.asarray(p['bef'],f32).sum(0)+np.asarray(p['aft'],f32).sum(0))/(2.0*N)
    sc[0,4]=(np.asarray(p['ba_tb'],f32)@bc1+np.asarray(p['ba_pb'],f32)@bc2
             +cs@(np.asarray(p['ba_pw'],f32).T@bc2))
    out['sc']=sc
    out['xv']=np.ascontiguousarray(xv.reshape(KT,128,6).transpose(1,0,2)).astype(BF)
    vps=np.stack([np.asarray(p['pr_pw'][j],f32).T@np.asarray(p['pr_cw'][j,INTER:],f32)
                  for j in range(3)],1)
    out['vp']=np.ascontiguousarray(vps.reshape(KT,128,3).transpose(1,0,2)).astype(BF)
    out['mgw']=np.ascontiguousarray(
        np.asarray(p['m_gw'],f32).T.reshape(KT,128,INTER).transpose(1,0,2)).astype(BF)
    prw=np.stack([np.asarray(p['pr_gw'][j],f32).T for j in range(3)])
    out['prw']=np.ascontiguousarray(
        prw.reshape(3,KT,128,INTER).transpose(2,0,1,3)).astype(BF)
    bg=float(np.asarray(p['ba_g'],f32)[0]); sg=float(np.asarray(p['sp_g'],f32)[0])
    gm=bg*(cs@np.asarray(p['ba_gw'],f32).T+np.asarray(p['ba_gb'],f32))
    b6=np.zeros((INTER,6),f32)
    b6[:,1:4]=np.asarray(p['pr_gb'],f32).T; b6[:,4]=gm
    b6[:,5]=sg*np.asarray(p['sp_gb'],f32)
    out['b6']=b6
    g=np.transpose(np.asarray(p['sp_gw'],f32),(2,3,1,0))[::-1,::-1]
    spg=sg*np.ascontiguousarray(g).reshape(9*GC,INTER)
    out['spg']=np.ascontiguousarray(spg.reshape(18,128,INTER).transpose(1,0,2)).astype(BF)
    we=np.einsum('c,cikl->ikl',np.asarray(p['sp_cw'][INTER:],f32),np.asarray(p['sp_pw'],f32))
    out['wef']=np.ascontiguousarray(we.reshape(2,128,9).transpose(1,0,2)).astype(BF)
    gf=np.zeros((1,4*INTER),f32)
    for j in range(3): gf[0,j*INTER:(j+1)*INTER]=np.asarray(p['pr_g'],f32)[j]
    gf[0,3*INTER:]=1.0
    out['gf']=gf.astype(BF)
    out['mgbr']=np.asarray(p['m_gb'],f32)[None,:].astype(BF)
    return out

def _shard(p):
    f32=np.float32
    gpadded=np.pad(np.asarray(p['global_feature'],f32)[0],((0,0),(1,1),(1,1)))
    x=np.asarray(p['origin_feature'],f32)
    ys=[np.asarray(p[t],f32) for t in ('local_feature','bef_l','aft_l')]
    ins=[]
    for k in range(NCORES):
        d={}
        rs=slice(k*NB,(k+1)*NB)
        d['xT']=np.ascontiguousarray(
            x[rs].T.reshape(KT,128,NB).transpose(1,0,2)).astype(BF)
        yb=np.stack([np.ascontiguousarray(y[rs].T).reshape(KT,128,NB) for y in ys])
        d['yT']=np.ascontiguousarray(yb.transpose(2,0,1,3)).astype(BF)
        gw=gpadded[:,k*HR:k*HR+WINR,:]                      # [GC,18,130]
        d['gps']=np.ascontiguousarray(
            gw.reshape(2,128,WIN).transpose(1,0,2)).astype(BF)
        gt=np.zeros((QT*128,GC),f32); gt[:WIN]=gw.reshape(GC,WIN).T
        d['gpt']=np.ascontiguousarray(
            gt.reshape(QT,128,GC).transpose(1,0,2)).astype(BF)
        ins.append(d)
    return ins

def kernel(**inputs):
    if 'nc' not in _cache:
        _cache['nc']=build()
    nc=_cache['nc']
    fold=_fold(inputs); shards=_shard(inputs)
    in_maps=[]
    for k in range(NCORES):
        m=dict(shards[k]); m.update(fold)
        in_maps.append({kk:np.ascontiguousarray(v) for kk,v in m.items()})
    from concourse.bass_utils import run_bass_kernel_spmd
    res=run_bass_kernel_spmd(nc,in_maps,list(range(NCORES)))
    out=np.empty((N,INTER),np.float32)
    for k in range(NCORES):
        out[k*NB:(k+1)*NB]=res.results[k]['out'].T
    return out


# ---- device program builder ----
import bass_rust
import concourse.bass as bass
import concourse.bacc as bacc
import concourse.mybir as mybir
import concourse.tile as tile

F32=mybir.dt.float32
BF16=mybir.dt.bfloat16
AF=mybir.ActivationFunctionType
AL=mybir.AluOpType
RG=[list(range(NCORES))]

def mkap(a,offset,dims):
    b=a.copy(); b.offset=offset
    b.ap=bass_rust.VecI64Pair([list(d) for d in dims])
    return b

def build():
    nc=bacc.Bacc("TRN2",target_bir_lowering=False,debug=False,num_devices=NCORES)
    def P(n,s,dt=BF16): return nc.declare_dram_parameter(n,list(s),dt,isOutput=False)
    xT=P('xT',(128,KT,NB)); yT=P('yT',(128,3,KT,NB))
    gps=P('gps',(128,2,WIN)); gpt=P('gpt',(128,QT,GC))
    xv=P('xv',(128,KT,6)); vp=P('vp',(128,KT,3)); mgw=P('mgw',(128,KT,INTER))
    prw=P('prw',(128,3,KT,INTER)); spg=P('spg',(128,18,INTER)); wef=P('wef',(128,2,9))
    b6=P('b6',(INTER,6),F32); gf=P('gf',(1,512)); sc=P('sc',(1,8),F32)
    mgbr=P('mgbr',(1,INTER))
    out_ext=nc.declare_dram_parameter('out',[INTER,NB],F32,isOutput=True)

    with tile.TileContext(nc) as tc:
      with (tc.tile_pool(name="pp",bufs=1) as pp,
            tc.tile_pool(name="ww",bufs=4) as ww,
            tc.tile_pool(name="dr",bufs=1,space="DRAM") as dr,
            tc.tile_pool(name="ps_or",bufs=1,space="PSUM") as ps_or,
            tc.tile_pool(name="ps_six",bufs=1,space="PSUM") as ps_six,
            tc.tile_pool(name="ps_mid",bufs=2,space="PSUM") as ps_mid,
            tc.tile_pool(name="ps_roll",bufs=3,space="PSUM") as ps_roll):
        ag_in=dr.tile([AGE],BF16); ag_out=dr.tile([NCORES*AGE],BF16,addr_space='Shared')
        ploc=dr.tile([2944],BF16)
        sdma=nc.sync.dma_start; cdma=nc.scalar.dma_start; vdma=nc.gpsimd.dma_start
        def ld(q,name,shape,src_ap,dt=BF16):
            t=pp.tile(shape,dt,tag=name,name=name)
            q(t[:],src_ap)
            return t
        # critical-path queue (sync)
        xv_s=ld(sdma,'xv',[128,KT,6],xv.ap())
        xT_s=ld(sdma,'xT',[128,KT,NB],xT.ap())
        mgw_s=ld(sdma,'mgw',[128,KT,INTER],mgw.ap())
        mgbr_s=ld(sdma,'mgbr',[1,INTER],mgbr.ap())
        gf_s=ld(sdma,'gf',[1,512],gf.ap())
        sc_s=ld(sdma,'sc',[1,8],sc.ap(),F32)
        b6_s=ld(sdma,'b6',[INTER,6],b6.ap(),F32)
        vp_s=ld(sdma,'vp',[128,KT,3],vp.ap())
        pr_s=ld(sdma,'pr',[128,3,KT,INTER],prw.ap())
        # conv/spatial queue (scalar)
        wef_s=ld(cdma,'wef',[128,2,9],wef.ap())
        gps_s=ld(cdma,'gps',[128,2,WIN],gps.ap())
        gpt_s=ld(cdma,'gpt',[128,QT,GC],gpt.ap())
        # pair tensors on gpsimd software queue; must clear before AG trigger
        yT_s=pp.tile([128,3,KT,NB],BF16,tag='yT')
        for j in range(3):
            vdma(yT_s[:,j,:,:],yT.ap()[:,j])
        ONESR=gf_s[0:1,3*INTER:4*INTER]
        ONE1=gf_s[0:1,3*INTER:3*INTER+1]
        zz=pp.tile([1,608],BF16,tag='zz'); nc.vector.memset(zz[:],0.0)
        ones16=pp.tile([16,1],F32,tag='ones16'); nc.vector.memset(ones16[:],1.0)
        onesf=pp.tile([1,INTER],F32,tag='onesf'); nc.vector.memset(onesf[:],1.0)
        e_sb=pp.tile([16,PW],BF16,tag='e_sb'); nc.vector.memset(e_sb[:],0.0)
        # ---- psum6: 6 folded x-dot-products ----
        p6=ps_six.tile([6,512],F32,tag='six')
        for kt in range(KT):
            nc.tensor.matmul(p6[:,:],xv_s[:,kt,:],xT_s[:,kt,:],start=(kt==0),
                             stop=(kt==KT-1))
        p6sb=pp.tile([6,512],F32,tag='p6sb')
        nc.scalar.activation(p6sb[:],p6[:,:],AF.Copy)
        p6r=[]
        for r in range(6):
            t=pp.tile([1,512],F32,tag=f'p6r{r}',name=f'p6r{r}')
            sdma(t[:],p6sb[r:r+1,:]); p6r.append(t)
        a_b=pp.tile([1,512],BF16,tag='a_b')
        nc.vector.tensor_scalar(a_b[:],p6r[0][:],sc_s[0:1,0:1],None,AL.add)
        b_b=pp.tile([1,512],BF16,tag='b_b')
        nc.vector.tensor_scalar(b_b[:],p6r[5][:],sc_s[0:1,5:6],None,AL.add)
        sdma(ag_in[BOFF:BOFF+NB],b_b[:])
        sba=pp.tile([1,512],BF16,tag='sba')
        nc.scalar.activation(sba[:],p6r[4][:],AF.Relu,bias=sc_s[0:1,4:5])
        # ---- g_x (own rows, row-major) ----
        gxo=pp.tile([128,4,INTER],BF16,tag='gxo')
        for i4 in range(4):
            pg=ps_mid.tile([128,512],F32,tag='mid')
            for kt in range(KT):
                nc.tensor.matmul(pg[:,:INTER],xT_s[:,kt,i4*128:(i4+1)*128],mgw_s[:,kt,:],
                                 start=(kt==0),stop=False,skip_group_check=True)
            nc.tensor.matmul(pg[:,:INTER],ONESR,mgbr_s[:],start=False,stop=True,
                             skip_group_check=True)
            nc.scalar.activation(gxo[:,i4,:],pg[:,:INTER],AF.Copy)
        sdma(mkap(ag_in[:],0,[(128,128),(16384,4),(1,128)]),gxo[:])
        # ---- conv -> b_s rows (own spatial window) ----
        outc=pp.tile([9,WIN],BF16,tag='outc')
        for ch in range(5):
            pc=ps_mid.tile([128,512],F32,tag='mid')
            for h in range(2):
                nc.tensor.matmul(pc[:9,:468],wef_s[:,h,:],gps_s[:,h,ch*468:(ch+1)*468],
                                 start=(h==0),stop=(h==1))
            nc.scalar.activation(outc[:,ch*468:(ch+1)*468],pc[:9,:468],AF.Copy)
        ov=outc[:].rearrange("p (h w) -> p h w",w=PW)
        bsa=pp.tile([HR,128],F32,tag='bsa')
        for m in range(9):
            kh,kw=divmod(m,3)
            bt=ww.tile([HR,128],BF16,tag='bt')
            cdma(bt[:],ov[m:m+1,kh:kh+HR,kw:kw+128])
            if m==0: nc.vector.tensor_copy(bsa[:],bt[:])
            else: nc.vector.tensor_tensor(bsa[:],bsa[:],bt[:],AL.add)
        zc=pp.tile([16,1],F32,tag='zc')
        nc.scalar.activation(e_sb[:,0:128],bsa[:],AF.Exp,accum_out=zc[:])
        pz=ps_roll.tile([128,512],F32,tag='roll')
        nc.tensor.matmul(pz[:1,:1],zc[:],ones16[:],start=True,stop=True)
        z_b=pp.tile([1,1],BF16,tag='z_b'); nc.vector.tensor_copy(z_b[:],pz[:1,:1])
        sdma(ag_in[ZOFF:ZOFF+1],z_b[:])
        # unnormalized p window -> ploc (guards zeroed), windowed lq gather
        cdma(ploc[0:262],zz[0:1,0:262])
        cdma(ploc[2342:2944],zz[0:1,0:602])
        cdma(mkap(ploc[:],262,[(130,16),(1,130)]),e_sb[:])
        lq_s=pp.tile([128,QT,9],BF16,tag='lq_s')
        for dr in range(3):
            cdma(lq_s[:,:,3*dr:3*dr+3],mkap(ploc[:],130*dr,[(1,128),(128,19),(1,3)]))
        spg_s=ld(cdma,'spg',[128,18,INTER],spg.ap())
        # ---- q correlation ----
        pq=ps_mid.tile([128,512],F32,tag='mid')
        for t in range(QT):
            nc.tensor.matmul(pq[:9,:GC],lq_s[:,t,:],gpt_s[:,t,:],start=(t==0),
                             stop=(t==QT-1))
        q_sb=pp.tile([9,GC],BF16,tag='q_sb')
        nc.scalar.activation(q_sb[:],pq[:9,:GC],AF.Copy)
        sdma(ag_in[QOFF:QOFF+2304],q_sb[:])
        # ---- a broadcast ----
        pab=ps_roll.tile([128,512],F32,tag='roll')
        nc.tensor.matmul(pab[:,:],ONESR,a_b[:],start=True,stop=True)
        ab_sb=pp.tile([128,512],BF16,tag='ab_sb')
        nc.scalar.activation(ab_sb[:],pab[:,:],AF.Copy)
        # ---- single collective ----
        nc.gpsimd.collective_compute("AllGather",AL.bypass,ins=[ag_in[:].opt()],
                                     outs=[ag_out[:].opt()],replica_groups=RG)
        # ---- during-collective: pair units + ba ----
        s_bs=[]
        for j in range(3):
            psv=ps_roll.tile([128,512],F32,tag='roll')
            for kt in range(KT):
                nc.tensor.matmul(psv[:1,:],vp_s[:,kt,j:j+1],yT_s[:,j,kt,:],
                                 start=(kt==0),stop=(kt==KT-1))
            spre=ww.tile([1,512],F32,tag='spre',bufs=2)
            nc.vector.tensor_scalar(spre[:],psv[:1,:],sc_s[0:1,1+j:2+j],None,AL.add)
            t2=ww.tile([1,512],F32,tag='t2',bufs=2)
            nc.vector.tensor_tensor(t2[:],p6r[1+j][:],spre[:],AL.add)
            sb=pp.tile([1,512],BF16,tag=f's_b{j}',name=f's_b{j}')
            nc.scalar.activation(sb[:],t2[:],AF.Relu); s_bs.append(sb)
        acc=pp.tile([128,512],F32,tag='acc')
        tmp=pp.tile([128,512],F32,tag='tmp')
        for j in range(3):
            py=ps_roll.tile([128,512],F32,tag='roll')
            for kt in range(KT):
                nc.tensor.matmul(py[:,:],pr_s[:,j,kt,:],yT_s[:,j,kt,:],
                                 start=(kt==0),stop=(kt==KT-1))
            gy=ww.tile([128,512],F32,tag='gy',bufs=2)
            nc.vector.tensor_scalar(gy[:],py[:,:],b6_s[:,1+j:2+j],None,AL.add)
            pb=ps_roll.tile([128,512],F32,tag='roll')
            nc.tensor.matmul(pb[:,:],gf_s[0:1,j*INTER:(j+1)*INTER],s_bs[j][:],
                             start=True,stop=True)
            if j==0:
                nc.vector.tensor_tensor(acc[:],gy[:],pb[:,:],AL.mult)
            else:
                nc.vector.tensor_tensor(tmp[:],gy[:],pb[:,:],AL.mult)
                nc.vector.tensor_tensor(acc[:],acc[:],tmp[:],AL.add)
        psb=ps_roll.tile([128,512],F32,tag='roll')
        nc.tensor.matmul(psb[:,:],ONESR,sba[:],start=True,stop=True)
        nc.vector.tensor_scalar(tmp[:],psb[:,:],b6_s[:,4:5],None,AL.mult)
        nc.vector.tensor_tensor(acc[:],acc[:],tmp[:],AL.add)
        # ---- post-collective readback ----
        bc_b=pp.tile([128,32],BF16,tag='bc_b')
        for c in range(NCORES):
            sdma(bc_b[:,4*c:4*c+4],mkap(ag_out[:],c*AGE+BOFF,[(1,128),(128,4)]))
        bc_s=pp.tile([128,32],F32,tag='bc_s')
        nc.vector.tensor_copy(bc_s[:],bc_b[:])
        gx_sb=pp.tile([128,32,128],BF16,tag='gx_sb')
        for c in range(NCORES):
            q=sdma if c%2==0 else cdma
            q(gx_sb[:,4*c:4*(c+1),:],mkap(ag_out[:],c*AGE,[(128,128),(16384,4),(1,128)]))
        q8=pp.tile([128,8,QT],BF16,tag='q8')
        for c in range(NCORES):
            cdma(q8[:,c,:],mkap(ag_out[:],c*AGE+QOFF,[(1,128),(128,19)]))
        # ---- main origin loop ----
        po=ps_or.tile([128,512],F32,tag='orig')
        for jt in range(32):
            fT=ww.tile([128,512],BF16,tag='fT',bufs=4)
            r=jt%16
            if r<5:
                nc.scalar.activation(fT[:],ab_sb[:],AF.Relu,bias=bc_s[:,jt:jt+1])
            elif r<9:
                nc.gpsimd.tensor_scalar(fT[:],ab_sb[:],bc_s[:,jt:jt+1],0.0,AL.add,AL.max)
            else:
                nc.vector.tensor_scalar(fT[:],ab_sb[:],bc_s[:,jt:jt+1],0.0,AL.add,AL.max)
            nc.tensor.matmul(po[:,:],gx_sb[:,jt,:],fT[:],start=(jt==0),stop=(jt==31),
                             skip_group_check=True)
        # ---- spatial v (needed only at the very end) ----
        s1=pp.tile([128,4,QT],BF16,tag='s1')
        nc.vector.tensor_tensor(s1[:],q8[:,0:4,:],q8[:,4:8,:],AL.add)
        s2=pp.tile([128,2,QT],BF16,tag='s2')
        nc.vector.tensor_tensor(s2[:],s1[:,0:2,:],s1[:,2:4,:],AL.add)
        qr=pp.tile([128,QT],BF16,tag='qr')
        nc.vector.tensor_tensor(qr[:],s2[:,0,:],s2[:,1,:],AL.add)
        zr=pp.tile([1,1],F32,tag='zr')
        nc.vector.reciprocal(zr[:],qr[0:1,18:19])
        pzb=ps_roll.tile([128,512],F32,tag='roll')
        nc.tensor.matmul(pzb[:,:1],onesf[:],zr[:],start=True,stop=True)
        zrb=pp.tile([128,1],F32,tag='zrb'); nc.vector.tensor_copy(zrb[:],pzb[:,:1])
        pvr=ps_roll.tile([128,512],F32,tag='roll')
        for t in range(18):
            nc.tensor.matmul(pvr[:1,:INTER],qr[:,t:t+1],spg_s[:,t,:],
                             start=(t==0),stop=(t==17))
        v_row=pp.tile([1,INTER],BF16,tag='v_row')
        nc.scalar.activation(v_row[:],pvr[:1,:INTER],AF.Copy)
        pvT=ps_roll.tile([128,512],F32,tag='roll')
        nc.tensor.matmul(pvT[:,:1],v_row[:],ONE1,start=True,stop=True)
        v_sb=pp.tile([128,1],F32,tag='v_sb')
        nc.vector.tensor_scalar(v_sb[:],pvT[:,:1],zrb[:],b6_s[:,5:6],AL.mult,AL.add)
        ot=pp.tile([128,512],F32,tag='ot')
        nc.vector.tensor_scalar(ot[:],po[:,:],1.0/N,v_sb[:],AL.mult,AL.add)
        fin=pp.tile([128,512],F32,tag='fin')
        nc.vector.tensor_tensor(fin[:],acc[:],ot[:],AL.add)
        sdma(out_ext.ap(),fin[:])
    nc.compile()
    return nc


# revision 15
# speedup vs baseline: 1.4315x; 1.4315x over previous
import numpy as np
import ml_dtypes

N=4096; C=1024; INTER=128; R=128; GC=256; NCORES=8; NB=N//NCORES
HR=R//NCORES; PW=130; WINR=HR+2; WIN=WINR*PW; QT=19; KT=C//128
GXE=NB*INTER; BOFF=GXE; QOFF=GXE+NB; ZOFF=QOFF+2304; AGE=QOFF+2432  # 68480
BF=ml_dtypes.bfloat16

_cache = {}

def _fold(p):
    f32=np.float32
    out={}
    mcw1=np.asarray(p['m_cw'][:INTER],f32); mcw2=np.asarray(p['m_cw'][INTER:],f32)
    xv=np.zeros((C,6),f32); sc=np.zeros((1,8),f32)
    xv[:,0]=np.asarray(p['m_tw'],f32).T@mcw1; sc[0,0]=np.asarray(p['m_tb'],f32)@mcw1
    for j in range(3):
        c1=np.asarray(p['pr_cw'][j,:INTER],f32); c2=np.asarray(p['pr_cw'][j,INTER:],f32)
        xv[:,1+j]=np.asarray(p['pr_tw'][j],f32).T@c1
        sc[0,1+j]=np.asarray(p['pr_tb'][j],f32)@c1+np.asarray(p['pr_pb'][j],f32)@c2
    bc1=np.asarray(p['ba_cw'][:INTER],f32); bc2=np.asarray(p['ba_cw'][INTER:],f32)
    xv[:,4]=np.asarray(p['ba_tw'],f32).T@bc1
    xv[:,5]=np.asarray(p['m_pw'],f32).T@mcw2; sc[0,5]=np.asarray(p['m_pb'],f32)@mcw2
    # bef/aft enter only via column means: fold on host
    cs=(np.asarray(p['bef'],f32).sum(0)+np.asarray(p['aft'],f32).sum(0))/(2.0*N)
    sc[0,4]=(np.asarray(p['ba_tb'],f32)@bc1+np.asarray(p['ba_pb'],f32)@bc2
             +cs@(np.asarray(p['ba_pw'],f32).T@bc2))
    out['sc']=sc
    out['xv']=np.ascontiguousarray(xv.reshape(KT,128,6).transpose(1,0,2)).astype(BF)
    vps=np.stack([np.asarray(p['pr_pw'][j],f32).T@np.asarray(p['pr_cw'][j,INTER:],f32)
                  for j in range(3)],1)
    out['vp']=np.ascontiguousarray(vps.reshape(KT,128,3).transpose(1,0,2)).astype(BF)
    out['mgw']=np.ascontiguousarray(
        np.asarray(p['m_gw'],f32).T.reshape(KT,128,INTER).transpose(1,0,2)).astype(BF)
    prw=np.stack([np.asarray(p['pr_gw'][j],f32).T for j in range(3)])
    out['prw']=np.ascontiguousarray(
        prw.reshape(3,KT,128,INTER).transpose(2,0,1,3)).astype(BF)
    bg=float(np.asarray(p['ba_g'],f32)[0]); sg=float(np.asarray(p['sp_g'],f32)[0])
    gm=bg*(cs@np.asarray(p['ba_gw'],f32).T+np.asarray(p['ba_gb'],f32))
    b6=np.zeros((INTER,6),f32)
    b6[:,1:4]=np.asarray(p['pr_gb'],f32).T; b6[:,4]=gm
    b6[:,5]=sg*np.asarray(p['sp_gb'],f32)
    out['b6']=b6
    g=np.transpose(np.asarray(p['sp_gw'],f32),(2,3,1,0))[::-1,::-1]
    spg=sg*np.ascontiguousarray(g).reshape(9*GC,INTER)
    out['spg']=np.ascontiguousarray(spg.reshape(18,128,INTER).transpose(1,0,2)).astype(BF)
    we=np.einsum('c,cikl->ikl',np.asarray(p['sp_cw'][INTER:],f32),np.asarray(p['sp_pw'],f32))
    out['wef']=np.ascontiguousarray(we.reshape(2,128,9).transpose(1,0,2)).astype(BF)
    gf=np.zeros((1,4*INTER),f32)
    for j in range(3): gf[0,j*INTER:(j+1)*INTER]=np.asarray(p['pr_g'],f32)[j]
    gf[0,3*INTER:]=1.0
    out['gf']=gf.astype(BF)
    out['mgbr']=np.asarray(p['m_gb'],f32)[None,:].astype(BF)
    return out

def _shard(p):
    f32=np.float32
    gpadded=np.pad(np.asarray(p['global_feature'],f32)[0],((0,0),(1,1),(1,1)))
    x=np.asarray(p['origin_feature'],f32)
    ys=[np.asarray(p[t],f32) for t in ('local_feature','bef_l','aft_l')]
    ins=[]
    for k in range(NCORES):
        d={}
        rs=slice(k*NB,(k+1)*NB)
        d['xT']=np.ascontiguousarray(
            x[rs].T.reshape(KT,128,NB).transpose(1,0,2)).astype(BF)
        yb=np.stack([np.ascontiguousarray(y[rs].T).reshape(KT,128,NB) for y in ys])
        d['yT']=np.ascontiguousarray(yb.transpose(2,0,1,3)).astype(BF)
        gw=gpadded[:,k*HR:k*HR+WINR,:]                      # [GC,18,130]
        d['gps']=np.ascontiguousarray(
            gw.reshape(2,128,WIN).transpose(1,0,2)).astype(BF)
        gt=np.zeros((QT*128,GC),f32); gt[:WIN]=gw.reshape(GC,WIN).T
        d['gpt']=np.ascontiguousarray(
            gt.reshape(QT,128,GC).transpose(1,0,2)).astype(BF)
        ins.append(d)
    return ins

def kernel(**inputs):
    if 'nc' not in _cache:
        _cache['nc']=build()
    nc=_cache['nc']
    fold=_fold(inputs); shards=_shard(inputs)
    in_maps=[]
    for k in range(NCORES):
        m=dict(shards[k]); m.update(fold)
        in_maps.append({kk:np.ascontiguousarray(v) for kk,v in m.items()})
    from concourse.bass_utils import run_bass_kernel_spmd
    res=run_bass_kernel_spmd(nc,in_maps,list(range(NCORES)))
    out=np.empty((N,INTER),np.float32)
    for k in range(NCORES):
        out[k*NB:(k+1)*NB]=res.results[k]['out'].T
    return out


# ---- device program builder ----
import bass_rust
import concourse.bass as bass
import concourse.bacc as bacc
import concourse.mybir as mybir
import concourse.tile as tile

F32=mybir.dt.float32
BF16=mybir.dt.bfloat16
AF=mybir.ActivationFunctionType
AL=mybir.AluOpType
RG=[list(range(NCORES))]

def mkap(a,offset,dims):
    b=a.copy(); b.offset=offset
    b.ap=bass_rust.VecI64Pair([list(d) for d in dims])
    return b

def build():
    nc=bacc.Bacc("TRN2",target_bir_lowering=False,debug=False,num_devices=NCORES)
    def P(n,s,dt=BF16): return nc.declare_dram_parameter(n,list(s),dt,isOutput=False)
    xT=P('xT',(128,KT,NB)); yT=P('yT',(128,3,KT,NB))
    gps=P('gps',(128,2,WIN)); gpt=P('gpt',(128,QT,GC))
    xv=P('xv',(128,KT,6)); vp=P('vp',(128,KT,3)); mgw=P('mgw',(128,KT,INTER))
    prw=P('prw',(128,3,KT,INTER)); spg=P('spg',(128,18,INTER)); wef=P('wef',(128,2,9))
    b6=P('b6',(INTER,6),F32); gf=P('gf',(1,512)); sc=P('sc',(1,8),F32)
    mgbr=P('mgbr',(1,INTER))
    out_ext=nc.declare_dram_parameter('out',[INTER,NB],F32,isOutput=True)

    with tile.TileContext(nc) as tc:
      with (tc.tile_pool(name="pp",bufs=1) as pp,
            tc.tile_pool(name="ww",bufs=4) as ww,
            tc.tile_pool(name="dr",bufs=1,space="DRAM") as dr,
            tc.tile_pool(name="ps_or",bufs=1,space="PSUM") as ps_or,
            tc.tile_pool(name="ps_six",bufs=1,space="PSUM") as ps_six,
            tc.tile_pool(name="ps_mid",bufs=2,space="PSUM") as ps_mid,
            tc.tile_pool(name="ps_roll",bufs=3,space="PSUM") as ps_roll):
        ag_in=dr.tile([AGE],BF16); ag_out=dr.tile([NCORES*AGE],BF16,addr_space='Shared')
        ploc=dr.tile([2944],BF16)
        sdma=nc.sync.dma_start; cdma=nc.scalar.dma_start; vdma=nc.gpsimd.dma_start
        def ld(q,name,shape,src_ap,dt=BF16):
            t=pp.tile(shape,dt,tag=name,name=name)
            q(t[:],src_ap)
            return t
        # critical-path queue (sync)
        xv_s=ld(sdma,'xv',[128,KT,6],xv.ap())
        xT_s=ld(sdma,'xT',[128,KT,NB],xT.ap())
        mgw_s=ld(sdma,'mgw',[128,KT,INTER],mgw.ap())
        mgbr_s=ld(sdma,'mgbr',[1,INTER],mgbr.ap())
        gf_s=ld(sdma,'gf',[1,512],gf.ap())
        sc_s=ld(sdma,'sc',[1,8],sc.ap(),F32)
        b6_s=ld(sdma,'b6',[INTER,6],b6.ap(),F32)
        vp_s=ld(sdma,'vp',[128,KT,3],vp.ap())
        pr_s=ld(sdma,'pr',[128,3,KT,INTER],prw.ap())
        # conv/spatial queue (scalar)
        wef_s=ld(cdma,'wef',[128,2,9],wef.ap())
        gps_s=ld(cdma,'gps',[128,2,WIN],gps.ap())
        gpt_s=ld(cdma,'gpt',[128,QT,GC],gpt.ap())
        # pair tensors on gpsimd software queue; must clear before AG trigger
        yT_s=pp.tile([128,3,KT,NB],BF16,tag='yT')
        for j in range(3):
            vdma(yT_s[:,j,:,:],yT.ap()[:,j])
        ONESR=gf_s[0:1,3*INTER:4*INTER]
        ONE1=gf_s[0:1,3*INTER:3*INTER+1]
        zz=pp.tile([1,608],BF16,tag='zz'); nc.vector.memset(zz[:],0.0)
        ones16=pp.tile([16,1],F32,tag='ones16'); nc.vector.memset(ones16[:],1.0)
        onesf=pp.tile([1,INTER],F32,tag='onesf'); nc.vector.memset(onesf[:],1.0)
        e_sb=pp.tile([16,PW],BF16,tag='e_sb'); nc.vector.memset(e_sb[:],0.0)
        # ---- psum6: 6 folded x-dot-products ----
        p6=ps_six.tile([6,512],F32,tag='six')
        for kt in range(KT):
            nc.tensor.matmul(p6[:,:],xv_s[:,kt,:],xT_s[:,kt,:],start=(kt==0),
                             stop=(kt==KT-1))
        p6sb=pp.tile([6,512],F32,tag='p6sb')
        nc.scalar.activation(p6sb[:],p6[:,:],AF.Copy)
        p6r=[]
        for r in range(6):
            t=pp.tile([1,512],F32,tag=f'p6r{r}',name=f'p6r{r}')
            sdma(t[:],p6sb[r:r+1,:]); p6r.append(t)
        a_b=pp.tile([1,512],BF16,tag='a_b')
        nc.vector.tensor_scalar(a_b[:],p6r[0][:],sc_s[0:1,0:1],None,AL.add)
        b_b=pp.tile([1,512],BF16,tag='b_b')
        nc.vector.tensor_scalar(b_b[:],p6r[5][:],sc_s[0:1,5:6],None,AL.add)
        sdma(ag_in[BOFF:BOFF+NB],b_b[:])
        sba=pp.tile([1,512],BF16,tag='sba')
        nc.scalar.activation(sba[:],p6r[4][:],AF.Relu,bias=sc_s[0:1,4:5])
        # ---- g_x (own rows, row-major) ----
        gxo=pp.tile([128,4,INTER],BF16,tag='gxo')
        for i4 in range(4):
            pg=ps_mid.tile([128,512],F32,tag='mid')
            for kt in range(KT):
                nc.tensor.matmul(pg[:,:INTER],xT_s[:,kt,i4*128:(i4+1)*128],mgw_s[:,kt,:],
                                 start=(kt==0),stop=False,skip_group_check=True)
            nc.tensor.matmul(pg[:,:INTER],ONESR,mgbr_s[:],start=False,stop=True,
                             skip_group_check=True)
            nc.scalar.activation(gxo[:,i4,:],pg[:,:INTER],AF.Copy)
        sdma(mkap(ag_in[:],0,[(128,128),(16384,4),(1,128)]),gxo[:])
        # ---- conv -> b_s rows (own spatial window) ----
        outc=pp.tile([9,WIN],BF16,tag='outc')
        for ch in range(5):
            pc=ps_mid.tile([128,512],F32,tag='mid')
            for h in range(2):
                nc.tensor.matmul(pc[:9,:468],wef_s[:,h,:],gps_s[:,h,ch*468:(ch+1)*468],
                                 start=(h==0),stop=(h==1))
            nc.scalar.activation(outc[:,ch*468:(ch+1)*468],pc[:9,:468],AF.Copy)
        ov=outc[:].rearrange("p (h w) -> p h w",w=PW)
        bsa=pp.tile([HR,128],F32,tag='bsa')
        for m in range(9):
            kh,kw=divmod(m,3)
            bt=ww.tile([HR,128],BF16,tag='bt')
            cdma(bt[:],ov[m:m+1,kh:kh+HR,kw:kw+128])
            if m==0: nc.vector.tensor_copy(bsa[:],bt[:])
            else: nc.vector.tensor_tensor(bsa[:],bsa[:],bt[:],AL.add)
        zc=pp.tile([16,1],F32,tag='zc')
        nc.scalar.activation(e_sb[:,0:128],bsa[:],AF.Exp,accum_out=zc[:])
        pz=ps_roll.tile([128,512],F32,tag='roll')
        nc.tensor.matmul(pz[:1,:1],zc[:],ones16[:],start=True,stop=True)
        z_b=pp.tile([1,1],BF16,tag='z_b'); nc.vector.tensor_copy(z_b[:],pz[:1,:1])
        sdma(ag_in[ZOFF:ZOFF+1],z_b[:])
        # unnormalized p window -> ploc (guards zeroed), windowed lq gather
        cdma(ploc[0:262],zz[0:1,0:262])
        cdma(ploc[2342:2944],zz[0:1,0:602])
        cdma(mkap(ploc[:],262,[(130,16),(1,130)]),e_sb[:])
        lq_s=pp.tile([128,QT,9],BF16,tag='lq_s')
        for dr in range(3):
            cdma(lq_s[:,:,3*dr:3*dr+3],mkap(ploc[:],130*dr,[(1,128),(128,19),(1,3)]))
        spg_s=ld(cdma,'spg',[128,18,INTER],spg.ap())
        # ---- q correlation ----
        pq=ps_mid.tile([128,512],F32,tag='mid')
        for t in range(QT):
            nc.tensor.matmul(pq[:9,:GC],lq_s[:,t,:],gpt_s[:,t,:],start=(t==0),
                             stop=(t==QT-1))
        q_sb=pp.tile([9,GC],BF16,tag='q_sb')
        nc.scalar.activation(q_sb[:],pq[:9,:GC],AF.Copy)
        sdma(ag_in[QOFF:QOFF+2304],q_sb[:])
        # ---- a broadcast ----
        pab=ps_roll.tile([128,512],F32,tag='roll')
        nc.tensor.matmul(pab[:,:],ONESR,a_b[:],start=True,stop=True)
        ab_sb=pp.tile([128,512],BF16,tag='ab_sb')
        nc.scalar.activation(ab_sb[:],pab[:,:],AF.Copy)
        # ---- single collective ----
        nc.gpsimd.collective_compute("AllGather",AL.bypass,ins=[ag_in[:].opt()],
                                     outs=[ag_out[:].opt()],replica_groups=RG)
        # ---- during-collective: pair units + ba ----
        s_bs=[]
        for j in range(3):
            psv=ps_roll.tile([128,512],F32,tag='roll')
            for kt in range(KT):
                nc.tensor.matmul(psv[:1,:],vp_s[:,kt,j:j+1],yT_s[:,j,kt,:],
                                 start=(kt==0),stop=(kt==KT-1))
            spre=ww.tile([1,512],F32,tag='spre',bufs=2)
            nc.vector.tensor_scalar(spre[:],psv[:1,:],sc_s[0:1,1+j:2+j],None,AL.add)
            t2=ww.tile([1,512],F32,tag='t2',bufs=2)
            nc.vector.tensor_tensor(t2[:],p6r[1+j][:],spre[:],AL.add)
            sb=pp.tile([1,512],BF16,tag=f's_b{j}',name=f's_b{j}')
            nc.scalar.activation(sb[:],t2[:],AF.Relu); s_bs.append(sb)
        acc=pp.tile([128,512],F32,tag='acc')
        tmp=pp.tile([128,512],F32,tag='tmp')
        for j in range(3):
            py=ps_roll.tile([128,512],F32,tag='roll')
            for kt in range(KT):
                nc.tensor.matmul(py[:,:],pr_s[:,j,kt,:],yT_s[:,j,kt,:],
                                 start=(kt==0),stop=(kt==KT-1))
            gy=ww.tile([128,512],F32,tag='gy',bufs=2)
            nc.vector.tensor_scalar(gy[:],py[:,:],b6_s[:,1+j:2+j],None,AL.add)
            pb=ps_roll.tile([128,512],F32,tag='roll')
            nc.tensor.matmul(pb[:,:],gf_s[0:1,j*INTER:(j+1)*INTER],s_bs[j][:],
                             start=True,stop=True)
            if j==0:
                nc.vector.tensor_tensor(acc[:],gy[:],pb[:,:],AL.mult)
            else:
                nc.vector.tensor_tensor(tmp[:],gy[:],pb[:,:],AL.mult)
                nc.vector.tensor_tensor(acc[:],acc[:],tmp[:],AL.add)
        psb=ps_roll.tile([128,512],F32,tag='roll')
        nc.tensor.matmul(psb[:,:],ONESR,sba[:],start=True,stop=True)
        nc.vector.tensor_scalar(tmp[:],psb[:,:],b6_s[:,4:5],None,AL.mult)
        nc.vector.tensor_tensor(acc[:],acc[:],tmp[:],AL.add)
        # ---- post-collective readback ----
        bc_b=pp.tile([128,32],BF16,tag='bc_b')
        for c in range(NCORES):
            sdma(bc_b[:,4*c:4*c+4],mkap(ag_out[:],c*AGE+BOFF,[(1,128),(128,4)]))
        bc_s=pp.tile([128,32],F32,tag='bc_s')
        nc.vector.tensor_copy(bc_s[:],bc_b[:])
        gx_sb=pp.tile([128,32,128],BF16,tag='gx_sb')
        for c in range(NCORES):
            q=sdma if c%2==0 else cdma
            q(gx_sb[:,4*c:4*(c+1),:],mkap(ag_out[:],c*AGE,[(128,128),(16384,4),(1,128)]))
        q8=pp.tile([128,8,QT],BF16,tag='q8')
        for c in range(NCORES):
            cdma(q8[:,c,:],mkap(ag_out[:],c*AGE+QOFF,[(1,128),(128,19)]))
        # ---- main origin loop ----
        po=ps_or.tile([128,512],F32,tag='orig')
        for jt in range(32):
            fT=ww.tile([128,512],BF16,tag='fT',bufs=4)
            if jt%8<3:
                nc.scalar.activation(fT[:],ab_sb[:],AF.Relu,bias=bc_s[:,jt:jt+1])
            else:
                nc.vector.tensor_scalar(fT[:],ab_sb[:],bc_s[:,jt:jt+1],0.0,AL.add,AL.max)
            nc.tensor.matmul(po[:,:],gx_sb[:,jt,:],fT[:],start=(jt==0),stop=(jt==31),
                             skip_group_check=True)
        # ---- spatial v (needed only at the very end) ----
        s1=pp.tile([128,4,QT],BF16,tag='s1')
        nc.vector.tensor_tensor(s1[:],q8[:,0:4,:],q8[:,4:8,:],AL.add)
        s2=pp.tile([128,2,QT],BF16,tag='s2')
        nc.vector.tensor_tensor(s2[:],s1[:,0:2,:],s1[:,2:4,:],AL.add)
        qr=pp.tile([128,QT],BF16,tag='qr')
        nc.vector.tensor_tensor(qr[:],s2[:,0,:],s2[:,1,:],AL.add)
        zr=pp.tile([1,1],F32,tag='zr')
        nc.vector.reciprocal(zr[:],qr[0:1,18:19])
        pzb=ps_roll.tile([128,512],F32,tag='roll')
        nc.tensor.matmul(pzb[:,:1],onesf[:],zr[:],start=True,stop=True)
        zrb=pp.tile([128,1],F32,tag='zrb'); nc.vector.tensor_copy(zrb[:],pzb[:,:1])
        pvr=ps_roll.tile([128,512],F32,tag='roll')
        for t in range(18):
            nc.tensor.matmul(pvr[:1,:INTER],qr[:,t:t+1],spg_s[:,t,:],
                             start=(t==0),stop=(t==17))
        v_row=pp.tile([1,INTER],BF16,tag='v_row')
        nc.scalar.activation(v_row[:],pvr[:1,:INTER],AF.Copy)
        pvT=ps_roll.tile([128,512],F32,tag='roll')
        nc.tensor.matmul(pvT[:,:1],v_row[:],ONE1,start=True,stop=True)
        v_sb=pp.tile([128,1],F32,tag='v_sb')
        nc.vector.tensor_scalar(v_sb[:],pvT[:,:1],zrb[:],b6_s[:,5:6],AL.mult,AL.add)
        ot=pp.tile([128,512],F32,tag='ot')
        nc.vector.tensor_scalar(ot[:],po[:,:],1.0/N,v_sb[:],AL.mult,AL.add)
        fin=pp.tile([128,512],F32,tag='fin')
        nc.vector.tensor_tensor(fin[:],acc[:],ot[:],AL.add)
        sdma(out_ext.ap(),fin[:])
    nc.compile()
    return nc
